# revision 26
# baseline (speedup 1.0000x reference)
"""Distributed Trainium2 kernel for ArticulatoryMetricLoss.

loss = mean_{i != j} ((||e_i||^2 + ||e_j||^2 - 2 e_i.e_j) - art_dist[i, j])^2

Strategy (8 NeuronCores), exploiting d2's symmetry:
  - 36 unordered 512x512 block-units (8 diag + 28 pairs) of the 8x8 grid.
    Each core runs 18 [128j x 512i] sub-jobs (14 off-diag covering both
    orientations at once, 4 diag), identical graph on every core; the host
    packs per-core buffers to select the blocks.
  - Gram in fp8(e4m3) with MatmulPerfMode.DoubleRow: 3 matmuls of K=256
    per sub-job (2x PE throughput vs bf16), psum[j, i] = q_j . q_i
    (UNSCALED, so the diag sub-jobs' stationary tiles are slices of the
    already-loaded moving tile - no separate DMA for them).
  - Both orientations of an off-diag unit merge analytically:
      (w-a1)^2 + (w-a2)^2 = 2 (w - abar)^2 + (a1-a2)^2/2,  abar=(a1+a2)/2
    The i-side norm folds into the art input on the host:
      a_eff = abar - (s_i - s_mean)   (fp8, range ~±50).
  - Per 2-bank psum pair, the device computes ONLY two parallel partial
    reductions (no u tensor, no dependency between them):
      DVE: Spa = sum_i p * a_eff     ACT: Sp2 = sum_i p^2
    Everything else is host algebra in f64 from the fp8-rounded inputs:
      d2 - abar = -(2p + a_eff) + c_j, so
      sum_i (d2 - abar)^2 = [4 Sp2 + 4 Spa] + sum_a2
                            - 2 c_j (2 Sp + sum_a) + 512 c_j^2
    with c_j = s_j + s_mean, Sp = q_j . Q_block (a host dot product),
    and sum_a/sum_a2 row sums of the rounded a_eff.

Numerics: fp8 quantization of E (host norms computed from the same
quantized values) gives ~1.3e-3 relative error on the final scalar.
Diagonal i==j terms are included ((a_ii)^2 each, ~3e-11 relative).
"""

import os
import sys
from contextlib import ExitStack

import numpy as np

for _p in ("/opt/trn_rl_repo", "/root/.axon_site/_ro/trn_rl_repo"):
    if os.path.isdir(_p) and _p not in sys.path:
        sys.path.insert(0, _p)

import ml_dtypes

import concourse.tile as tile
from concourse import bacc, mybir
from concourse.bass_utils import run_bass_kernel_spmd

B = 4096          # rows/cols of the pairwise matrix
D = 768           # embedding dim
NCORES = 8
BLK = 512         # i/j block size (8x8 block grid)
P = 128           # SBUF partitions
KC = 3            # DoubleRow k-chunks of 256 (D = 3*256)
NSUB = 18         # sub-jobs per core
NBATCH = 6        # sub-jobs per matmul batch
NRED = 10         # reduction groups: 8 psum pairs + 2 singles (t16, t17)
PAIRS = B * (B - 1)

F8 = mybir.dt.float8e4
BF16 = mybir.dt.bfloat16
F32 = mybir.dt.float32
DR = mybir.MatmulPerfMode.DoubleRow

# per-core sub-job emission order: t0..11 gap-1..3 off-diag (W=2, mov slot
# 0), t12..13 half of the gap-4 pair (W=2, mov slot 1), t14..17 diag (W=1,
# mov slot 0). Reduction groups are W-homogeneous: pairs (2q, 2q+1) for
# q<8, then singles t16, t17 (own psum tiles so the final reductions
# pipeline across DVE/ACT instead of chaining on one tile).
SUB_SLOT = [0] * 12 + [1] * 2 + [0] * 4
RED_W = [2] * 7 + [1] * 3
RED_SUBJOBS = [(2 * q, 2 * q + 1) for q in range(8)] + [(16,), (17,)]

# bulk input layout, bytes per partition (all fp8). Batch 1 is laid out
# for pair-major streaming: [mov0 k | pair0-stat k] interleaved per k, then
# pair1/pair2 stats k-major, stat b2, mov1, stat b3' (t12/t13 only; diag
# stationaries t14-17 alias into mov0).
MOVK = 2 * BLK            # one mov k-layer (1024)
SPK = 2 * 2 * P           # one pair-stat k-layer (512)
STK = NBATCH * 2 * P      # one stat batch k-layer (1536)
STB = KC * STK            # one stat batch (4608)
ST3K = 2 * 2 * P          # b3' k-layer: t12,t13 only (512)
OFF_P1 = KC * (MOVK + SPK)            # 4608: pair1 stats
OFF_P2 = OFF_P1 + KC * SPK            # 6144: pair2 stats
OFF_B2 = OFF_P2 + KC * SPK            # 7680
OFF_M1 = OFF_B2 + STB                 # 12288
OFF_B3 = OFF_M1 + KC * MOVK           # 15360
BULK = OFF_B3 + KC * ST3K             # 16896

_CACHED = {}


def subjobs(c):
    """Host-side per-core sub-job table: (bi, bj, jt, W). bi must follow
    SUB_SLOT: block c for slot 0, block c%4 for slot 1."""
    jobs = []
    for d in (1, 2, 3):
        for jt in range(4):
            jobs.append((c, (c + d) % 8, jt, 2))
    p = c % 4
    for q in range(2):
        jt = q if c < 4 else q + 2
        jobs.append((p, p + 4, jt, 2))
    for jt in range(4):
        jobs.append((c, c, jt, 1))
    return jobs


def build_graph():
    nc = bacc.Bacc("TRN2", target_bir_lowering=False, debug=False, num_devices=NCORES)

    bulk_d = nc.dram_tensor("bulk", [P, BULK], F8, kind="ExternalInput")
    art_d = nc.dram_tensor("art", [P, NSUB * BLK], F8, kind="ExternalInput")
    sp2_d = nc.dram_tensor("sp2", [P, NRED], F32, kind="ExternalOutput")
    spa_d = nc.dram_tensor("spa", [P, NRED], F32, kind="ExternalOutput")

    with tile.TileContext(nc) as tc, ExitStack() as ctx:
        bulk_pool = ctx.enter_context(tc.tile_pool(name="bulk", bufs=1))
        art_pool = ctx.enter_context(tc.tile_pool(name="art", bufs=1))
        scr_pool = ctx.enter_context(tc.tile_pool(name="scr", bufs=2))
        sq_pool = ctx.enter_context(tc.tile_pool(name="sq", bufs=2))
        acc_pool = ctx.enter_context(tc.tile_pool(name="acc", bufs=1))
        psum_pool = ctx.enter_context(tc.tile_pool(name="psum", bufs=3, space="PSUM"))
        pss_pool = ctx.enter_context(tc.tile_pool(name="pss", bufs=2, space="PSUM"))

        bulk_t = bulk_pool.tile([P, BULK], F8)
        art_t = art_pool.tile([P, NSUB * BLK], F8)

        # ACT Square table prewarm (first use otherwise pays ~1.3us mid-run)
        warm = acc_pool.tile([P, 4], BF16, name="actwarm")
        nc.scalar.activation(
            warm[0:1, 0:1], warm[0:1, 1:2], mybir.ActivationFunctionType.Square
        )

        # ---- all loads on one sync (HWDGE) queue, ordered by first use:
        # batch-1 stat/mov pieces, then art interleaved just ahead of the
        # DVE/ACT consumers so their chains start as soon as psum is ready.
        def ld(dst, src, a, b):
            nc.sync.dma_start(dst[:, a:b], src[:, a:b])

        ld(bulk_t, bulk_d, 0, MOVK + SPK)
        ld(bulk_t, bulk_d, MOVK + SPK, OFF_P1)
        ld(bulk_t, bulk_d, OFF_P1, OFF_B2)
        ld(art_t, art_d, 0, 4 * BLK)              # art pairs 0-1
        ld(bulk_t, bulk_d, OFF_B2, OFF_M1)        # stat b2
        ld(art_t, art_d, 4 * BLK, 10 * BLK)       # art pairs 2-4
        ld(bulk_t, bulk_d, OFF_M1, BULK)          # mov1 + stat b3'
        ld(art_t, art_d, 10 * BLK, NSUB * BLK)    # art pairs 5-7 + singles

        def stat_view(t, k):  # [128, 2, 128] stationary for sub-job t, chunk k
            b, ti = divmod(t, NBATCH)
            if b == 0:
                if ti < 2:
                    off = MOVK + k * (MOVK + SPK) + ti * 2 * P
                else:
                    off = (OFF_P1 if ti < 4 else OFF_P2) + k * SPK + (ti % 2) * 2 * P
            elif b == 1:
                off = OFF_B2 + k * STK + ti * 2 * P
            elif t >= 14:   # diag: slice of the mov0 k-layer (same fp8 data)
                jt = t - 14
                return (
                    bulk_t[:, k * (MOVK + SPK) : k * (MOVK + SPK) + MOVK]
                    .rearrange("p (two n) -> p two n", two=2)[:, :, jt * P : (jt + 1) * P]
                )
            else:           # t12, t13 (ti is 0/1 within batch 3)
                off = OFF_B3 + k * ST3K + ti * 2 * P
            return bulk_t[:, off : off + 2 * P].rearrange(
                "p (two m) -> p two m", two=2
            )

        def mov_view(s, k):  # [128, 2, 512] moving for slot s, chunk k
            off = k * (MOVK + SPK) if s == 0 else OFF_M1 + k * MOVK
            return bulk_t[:, off : off + MOVK].rearrange(
                "p (two n) -> p two n", two=2
            )

        SP2 = acc_pool.tile([P, NRED], F32)
        SPA = acc_pool.tile([P, NRED], F32)

        # ---- pair-major: each reduction group's matmuls run consecutively,
        # so groups complete evenly and DVE/ACT stream without bursts.
        for rq, ts in enumerate(RED_SUBJOBS):
            width = len(ts) * BLK
            if len(ts) == 2:
                ps = psum_pool.tile([P, width], F32, tag="ps", name=f"ps{rq}")
            else:
                ps = pss_pool.tile([P, width], F32, tag="pss", name=f"ps{rq}")
            for k in range(KC):
                for half, t in enumerate(ts):
                    nc.tensor.matmul(
                        ps[:, half * BLK : (half + 1) * BLK],
                        stat_view(t, k),
                        mov_view(SUB_SLOT[t], k),
                        start=(k == 0),
                        stop=(k == KC - 1),
                        perf_mode=DR,
                    )
            av = art_t[:, ts[0] * BLK : ts[0] * BLK + width]
            pa = scr_pool.tile([P, 2 * BLK], BF16, tag="pa", name=f"pa{rq}")
            nc.vector.scalar_tensor_tensor(   # Spa = sum p * a_eff
                out=pa[:, :width],
                in0=ps[:],
                scalar=0.0,
                in1=av,
                op0=mybir.AluOpType.add,
                op1=mybir.AluOpType.mult,
                accum_out=SPA[:, rq : rq + 1],
            )
            sq = sq_pool.tile([P, 2 * BLK], BF16, tag="sq", name=f"sq{rq}")
            nc.scalar.activation(             # Sp2 = sum p^2
                sq[:, :width],
                ps[:],
                mybir.ActivationFunctionType.Square,
                accum_out=SP2[:, rq : rq + 1],
            )

        # outputs on separate queues; spa (DVE, finishes first) on sync
        nc.sync.dma_start(spa_d[:], SPA[:])
        nc.scalar.dma_start(sp2_d[:], SP2[:])

    nc.compile()
    return nc


def shard_inputs(embeddings: np.ndarray, art_dist: np.ndarray):
    """Pack per-core device inputs; also return host-side correction data."""
    f8 = ml_dtypes.float8_e4m3
    q8 = embeddings.astype(f8)                     # device-exact fp8 E
    qf = q8.astype(np.float64)
    s = (qf * qf).sum(axis=1)                      # norms of quantized E, f64
    sbar = s.mean()
    qblk = qf.reshape(8, BLK, D)
    Qsum = qblk.sum(axis=1)                        # [8, 768] block sums

    in_maps = []
    host = {"delta": 0.0, "cores": []}
    for c in range(NCORES):
        jobs = subjobs(c)
        bulk = np.empty((P, BULK), dtype=f8)
        # mov slots (unscaled q): [k][g][i]
        movs = []
        for bl in (c, c % 4):
            mT = np.ascontiguousarray(q8[bl * BLK : (bl + 1) * BLK].T)
            movs.append(mT.reshape(KC, 2, P, BLK).transpose(2, 0, 1, 3))
        for k in range(KC):
            bulk[:, k * (MOVK + SPK) : k * (MOVK + SPK) + MOVK] = movs[0][
                :, k
            ].reshape(P, MOVK)
            bulk[:, OFF_M1 + k * MOVK : OFF_M1 + (k + 1) * MOVK] = movs[1][
                :, k
            ].reshape(P, MOVK)
        # stat batches: [k][t][g][j]; batch 1 split per pair; b3 holds only
        # t12/t13 (diag aliases mov0)
        def stat_arr(tsel):
            sl = [
                np.ascontiguousarray(
                    q8.T[:, bj * BLK + jt * P : bj * BLK + (jt + 1) * P]
                ).reshape(KC, 2, P, P)
                for (bi, bj, jt, W) in (jobs[t] for t in tsel)
            ]
            return np.stack(sl).transpose(3, 1, 0, 2, 4)  # [p, k, t, g, j]
        a0 = stat_arr(range(0, 2))
        for k in range(KC):
            off = MOVK + k * (MOVK + SPK)
            bulk[:, off : off + SPK] = a0[:, k].reshape(P, SPK)
        for pi, tsel in ((0, range(2, 4)), (1, range(4, 6))):
            ap = stat_arr(tsel)
            base = OFF_P1 + pi * KC * SPK
            for k in range(KC):
                bulk[:, base + k * SPK : base + (k + 1) * SPK] = ap[:, k].reshape(
                    P, SPK
                )
        bulk[:, OFF_B2 : OFF_B2 + STB] = stat_arr(range(6, 12)).reshape(P, STB)
        bulk[:, OFF_B3 :] = stat_arr(range(12, 14)).reshape(P, KC * ST3K)
        # art_eff tiles + host corrections
        tiles = []
        corr = 0.0
        for (bi, bj, jt, W) in jobs:
            I = slice(bi * BLK, (bi + 1) * BLK)
            J = slice(bj * BLK + jt * P, bj * BLK + (jt + 1) * P)
            a_ji = art_dist[J, I].astype(np.float64)        # [128, 512]
            if W == 2:
                a_ij = art_dist[I, J].astype(np.float64).T  # [128, 512]
                abar = 0.5 * (a_ij + a_ji)
                dd = 0.5 * (a_ij - a_ji)
                host["delta"] += 2.0 * float((dd * dd).sum())
            else:
                abar = a_ji
            aef8 = (abar - (s[I] - sbar)[None, :]).astype(f8)
            tiles.append(aef8)
            aef = aef8.astype(np.float64)
            Sa = aef.sum(axis=1)                             # [128]
            Sa2 = (aef * aef).sum(axis=1)
            Sp = qf[J] @ Qsum[bi]                            # sum_i p per j
            cj = s[J] + sbar
            corr += W * float(
                (Sa2 - 2.0 * cj * (2.0 * Sp + Sa) + BLK * cj * cj).sum()
            )
        art = np.concatenate(tiles, axis=1)
        host["cores"].append(corr)
        in_maps.append(
            {"bulk": bulk, "art": np.ascontiguousarray(art)}
        )
    return in_maps, host


def combine(results, host):
    total = host["delta"]
    for c in range(NCORES):
        sp2 = results[c]["sp2"].astype(np.float64)   # [128, 10]
        spa = results[c]["spa"].astype(np.float64)
        dev = 4.0 * (sp2 + spa)
        for rq in range(NRED):
            total += RED_W[rq] * float(dev[:, rq].sum())
        total += host["cores"][c]
    return np.float32(total / PAIRS)


def _get_nc():
    if "nc" not in _CACHED:
        _CACHED["nc"] = build_graph()
    return _CACHED["nc"]


def _ensure_ntff_hook():
    """The agent image's antenv package lacks axon_hooks, so trace=True in
    run_bass_kernel_spmd crashes on import. Recreate the module + register
    the ctypes NTFF hook the way trn_boot would have."""
    try:
        from antenv.axon_hooks import get_axon_ntff_profile_hook  # noqa: F401

        return
    except ImportError:
        pass
    import types

    import antenv

    mod = types.ModuleType("antenv.axon_hooks")
    holder = {"hook": None}
    mod.set_axon_ntff_profile_hook = lambda h: holder.__setitem__("hook", h)
    mod.get_axon_ntff_profile_hook = lambda: holder["hook"]
    sys.modules["antenv.axon_hooks"] = mod
    antenv.axon_hooks = mod
    try:
        from trn_agent_boot.trn_boot import _ntff_profile_via_ctypes

        for so in ("/opt/axon/libaxon_pjrt.so",):
            if os.path.exists(so):
                holder["hook"] = _ntff_profile_via_ctypes(so)
                break
    except Exception as e:  # degrade: tracing skipped, run still works
        print(f"ntff hook setup failed ({e}); tracing disabled", file=sys.stderr)


def run(embeddings: np.ndarray, art_dist: np.ndarray, **run_kwargs):
    if run_kwargs.get("trace"):
        _ensure_ntff_hook()
    nc = _get_nc()
    in_maps, host = shard_inputs(np.asarray(embeddings), np.asarray(art_dist))
    res = run_bass_kernel_spmd(nc, in_maps, core_ids=list(range(NCORES)), **run_kwargs)
    loss = combine(res.results, host)
    return np.asarray(loss, dtype=np.float32), res


def kernel(embeddings: np.ndarray, art_dist: np.ndarray) -> np.ndarray:
    loss, _ = run(embeddings, art_dist)
    return loss


# revision 27
# speedup vs baseline: 1.1901x; 1.1901x over previous
"""Distributed Trainium2 kernel for ArticulatoryMetricLoss.

loss = mean_{i != j} ((||e_i||^2 + ||e_j||^2 - 2 e_i.e_j) - art_dist[i, j])^2

Strategy (8 NeuronCores), exploiting d2's symmetry:
  - 36 unordered 512x512 block-units (8 diag + 28 pairs) of the 8x8 grid.
    Each core runs 18 [128j x 512i] sub-jobs (14 off-diag covering both
    orientations at once, 4 diag), identical graph on every core; the host
    packs per-core buffers to select the blocks.
  - Gram in fp8(e4m3) with MatmulPerfMode.DoubleRow: 3 matmuls of K=256
    per sub-job (2x PE throughput vs bf16), psum[j, i] = q_j . q_i
    (UNSCALED, so the diag sub-jobs' stationary tiles are slices of the
    already-loaded moving tile - no separate DMA for them).
  - Both orientations of an off-diag unit merge analytically:
      (w-a1)^2 + (w-a2)^2 = 2 (w - abar)^2 + (a1-a2)^2/2,  abar=(a1+a2)/2
    The i-side norm folds into the art input on the host:
      a_eff = abar - (s_i - s_mean)   (fp8, range ~±50).
  - Per 2-bank psum pair, the device computes ONLY two parallel partial
    reductions (no u tensor, no dependency between them):
      DVE: Spa = sum_i p * a_eff     ACT: Sp2 = sum_i p^2
    Everything else is host algebra in f64 from the fp8-rounded inputs:
      d2 - abar = -(2p + a_eff) + c_j, so
      sum_i (d2 - abar)^2 = [4 Sp2 + 4 Spa] + sum_a2
                            - 2 c_j (2 Sp + sum_a) + 512 c_j^2
    with c_j = s_j + s_mean, Sp = q_j . Q_block (a host dot product),
    and sum_a/sum_a2 row sums of the rounded a_eff.

Numerics: fp8 quantization of E (host norms computed from the same
quantized values) gives ~1.3e-3 relative error on the final scalar.
Diagonal i==j terms are included ((a_ii)^2 each, ~3e-11 relative).
"""

import os
import sys
from contextlib import ExitStack

import numpy as np

for _p in ("/opt/trn_rl_repo", "/root/.axon_site/_ro/trn_rl_repo"):
    if os.path.isdir(_p) and _p not in sys.path:
        sys.path.insert(0, _p)

import ml_dtypes

import concourse.tile as tile
from concourse import bacc, mybir
from concourse.bass_utils import run_bass_kernel_spmd

B = 4096          # rows/cols of the pairwise matrix
D = 768           # embedding dim
NCORES = 8
BLK = 512         # i/j block size (8x8 block grid)
P = 128           # SBUF partitions
KC = 3            # DoubleRow k-chunks of 256 (D = 3*256)
NSUB = 18         # sub-jobs per core
NBATCH = 6        # sub-jobs per matmul batch
NPAIR = NSUB // 2
PAIRS = B * (B - 1)

F8 = mybir.dt.float8e4
BF16 = mybir.dt.bfloat16
F32 = mybir.dt.float32
DR = mybir.MatmulPerfMode.DoubleRow

# per-core sub-job emission order: t0..11 gap-1..3 off-diag (W=2, mov slot
# 0), t12..13 half of the gap-4 pair (W=2, mov slot 1), t14..17 diag (W=1,
# mov slot 0). Psum pairs (t, t+1) are W-homogeneous.
SUB_SLOT = [0] * 12 + [1] * 2 + [0] * 4
PAIR_W = [2] * 7 + [1] * 2

# bulk input layout, bytes per partition (all fp8). Batch 1 is laid out
# for pair-major streaming: [mov0 k | pair0-stat k] interleaved per k, then
# pair1/pair2 stats k-major, stat b2, mov1, stat b3' (t12/t13 only; diag
# stationaries t14-17 alias into mov0).
MOVK = 2 * BLK            # one mov k-layer (1024)
SPK = 2 * 2 * P           # one pair-stat k-layer (512)
STK = NBATCH * 2 * P      # one stat batch k-layer (1536)
STB = KC * STK            # one stat batch (4608)
ST3K = 2 * 2 * P          # b3' k-layer: t12,t13 only (512)
OFF_P1 = KC * (MOVK + SPK)            # 4608: pair1 stats
OFF_P2 = OFF_P1 + KC * SPK            # 6144: pair2 stats
OFF_B2 = OFF_P2 + KC * SPK            # 7680
OFF_M1 = OFF_B2 + STB                 # 12288
OFF_B3 = OFF_M1 + KC * MOVK           # 15360
BULK = OFF_B3 + KC * ST3K             # 16896

_CACHED = {}


def subjobs(c):
    """Host-side per-core sub-job table: (bi, bj, jt, W). bi must follow
    SUB_SLOT: block c for slot 0, block c%4 for slot 1."""
    jobs = []
    for d in (1, 2, 3):
        for jt in range(4):
            jobs.append((c, (c + d) % 8, jt, 2))
    p = c % 4
    for q in range(2):
        jt = q if c < 4 else q + 2
        jobs.append((p, p + 4, jt, 2))
    for jt in range(4):
        jobs.append((c, c, jt, 1))
    return jobs


def build_graph():
    nc = bacc.Bacc("TRN2", target_bir_lowering=False, debug=False, num_devices=NCORES)

    bulk_d = nc.dram_tensor("bulk", [P, BULK], F8, kind="ExternalInput")
    art_d = nc.dram_tensor("art", [P, NSUB * BLK], F8, kind="ExternalInput")
    sp2_d = nc.dram_tensor("sp2", [P, NPAIR], F32, kind="ExternalOutput")
    spa_d = nc.dram_tensor("spa", [P, NPAIR], F32, kind="ExternalOutput")

    with tile.TileContext(nc) as tc, ExitStack() as ctx:
        bulk_pool = ctx.enter_context(tc.tile_pool(name="bulk", bufs=1))
        art_pool = ctx.enter_context(tc.tile_pool(name="art", bufs=1))
        scr_pool = ctx.enter_context(tc.tile_pool(name="scr", bufs=2))
        sq_pool = ctx.enter_context(tc.tile_pool(name="sq", bufs=2))
        acc_pool = ctx.enter_context(tc.tile_pool(name="acc", bufs=1))
        psum_pool = ctx.enter_context(tc.tile_pool(name="psum", bufs=4, space="PSUM"))

        bulk_t = bulk_pool.tile([P, BULK], F8)
        art_t = art_pool.tile([P, NSUB * BLK], F8)

        # ACT Square table prewarm (first use otherwise pays ~1.3us mid-run)
        warm = acc_pool.tile([P, 4], BF16, name="actwarm")
        nc.scalar.activation(
            warm[0:1, 0:1], warm[0:1, 1:2], mybir.ActivationFunctionType.Square
        )

        # ---- all loads on one sync (HWDGE) queue, ordered by first use so
        # art never competes with the PE-critical stat/mov stream early on.
        nc.sync.dma_start(bulk_t[:, : MOVK + SPK], bulk_d[:, : MOVK + SPK])
        nc.sync.dma_start(bulk_t[:, MOVK + SPK : OFF_P1], bulk_d[:, MOVK + SPK : OFF_P1])
        nc.sync.dma_start(bulk_t[:, OFF_P1 : OFF_B2], bulk_d[:, OFF_P1 : OFF_B2])
        nc.sync.dma_start(bulk_t[:, OFF_B2 : OFF_M1], bulk_d[:, OFF_B2 : OFF_M1])
        nc.sync.dma_start(art_t[:, : NBATCH * BLK], art_d[:, : NBATCH * BLK])
        nc.sync.dma_start(bulk_t[:, OFF_M1 :], bulk_d[:, OFF_M1 :])
        nc.sync.dma_start(art_t[:, NBATCH * BLK :], art_d[:, NBATCH * BLK :])

        def stat_view(t, k):  # [128, 2, 128] stationary for sub-job t, chunk k
            b, ti = divmod(t, NBATCH)
            if b == 0:
                if ti < 2:
                    off = MOVK + k * (MOVK + SPK) + ti * 2 * P
                else:
                    off = (OFF_P1 if ti < 4 else OFF_P2) + k * SPK + (ti % 2) * 2 * P
            elif b == 1:
                off = OFF_B2 + k * STK + ti * 2 * P
            elif t >= 14:   # diag: slice of the mov0 k-layer (same fp8 data)
                jt = t - 14
                return (
                    bulk_t[:, k * (MOVK + SPK) : k * (MOVK + SPK) + MOVK]
                    .rearrange("p (two n) -> p two n", two=2)[:, :, jt * P : (jt + 1) * P]
                )
            else:           # t12, t13 (ti is 0/1 within batch 3)
                off = OFF_B3 + k * ST3K + ti * 2 * P
            return bulk_t[:, off : off + 2 * P].rearrange(
                "p (two m) -> p two m", two=2
            )

        def mov_view(s, k):  # [128, 2, 512] moving for slot s, chunk k
            off = k * (MOVK + SPK) if s == 0 else OFF_M1 + k * MOVK
            return bulk_t[:, off : off + MOVK].rearrange(
                "p (two n) -> p two n", two=2
            )

        SP2 = acc_pool.tile([P, NPAIR], F32)
        SPA = acc_pool.tile([P, NPAIR], F32)

        # ---- pair-major: each psum pair's 6 matmuls run consecutively, so
        # pairs complete evenly and DVE/ACT stream without bursts.
        for pq in range(NPAIR):
            t0 = 2 * pq
            ps = psum_pool.tile([P, 2 * BLK], F32, tag="ps", name=f"ps{pq}")
            for k in range(KC):
                for half, t in enumerate((t0, t0 + 1)):
                    nc.tensor.matmul(
                        ps[:, half * BLK : (half + 1) * BLK],
                        stat_view(t, k),
                        mov_view(SUB_SLOT[t], k),
                        start=(k == 0),
                        stop=(k == KC - 1),
                        perf_mode=DR,
                    )
            av = art_t[:, pq * 2 * BLK : (pq + 1) * 2 * BLK]
            pa = scr_pool.tile([P, 2 * BLK], BF16, tag="pa", name=f"pa{pq}")
            nc.vector.scalar_tensor_tensor(   # Spa = sum p * a_eff
                out=pa[:],
                in0=ps[:],
                scalar=0.0,
                in1=av,
                op0=mybir.AluOpType.add,
                op1=mybir.AluOpType.mult,
                accum_out=SPA[:, pq : pq + 1],
            )
            sq = sq_pool.tile([P, 2 * BLK], BF16, tag="sq", name=f"sq{pq}")
            nc.scalar.activation(             # Sp2 = sum p^2
                sq[:],
                ps[:],
                mybir.ActivationFunctionType.Square,
                accum_out=SP2[:, pq : pq + 1],
            )

        # outputs on separate queues; spa (DVE, finishes first) on sync
        nc.sync.dma_start(spa_d[:], SPA[:])
        nc.scalar.dma_start(sp2_d[:], SP2[:])

    nc.compile()
    return nc


def shard_inputs(embeddings: np.ndarray, art_dist: np.ndarray):
    """Pack per-core device inputs; also return host-side correction data."""
    f8 = ml_dtypes.float8_e4m3
    q8 = embeddings.astype(f8)                     # device-exact fp8 E
    qf = q8.astype(np.float64)
    s = (qf * qf).sum(axis=1)                      # norms of quantized E, f64
    sbar = s.mean()
    qblk = qf.reshape(8, BLK, D)
    Qsum = qblk.sum(axis=1)                        # [8, 768] block sums

    in_maps = []
    host = {"delta": 0.0, "cores": []}
    for c in range(NCORES):
        jobs = subjobs(c)
        bulk = np.empty((P, BULK), dtype=f8)
        # mov slots (unscaled q): [k][g][i]
        movs = []
        for bl in (c, c % 4):
            mT = np.ascontiguousarray(q8[bl * BLK : (bl + 1) * BLK].T)
            movs.append(mT.reshape(KC, 2, P, BLK).transpose(2, 0, 1, 3))
        for k in range(KC):
            bulk[:, k * (MOVK + SPK) : k * (MOVK + SPK) + MOVK] = movs[0][
                :, k
            ].reshape(P, MOVK)
            bulk[:, OFF_M1 + k * MOVK : OFF_M1 + (k + 1) * MOVK] = movs[1][
                :, k
            ].reshape(P, MOVK)
        # stat batches: [k][t][g][j]; batch 1 split per pair; b3 holds only
        # t12/t13 (diag aliases mov0)
        def stat_arr(tsel):
            sl = [
                np.ascontiguousarray(
                    q8.T[:, bj * BLK + jt * P : bj * BLK + (jt + 1) * P]
                ).reshape(KC, 2, P, P)
                for (bi, bj, jt, W) in (jobs[t] for t in tsel)
            ]
            return np.stack(sl).transpose(3, 1, 0, 2, 4)  # [p, k, t, g, j]
        a0 = stat_arr(range(0, 2))
        for k in range(KC):
            off = MOVK + k * (MOVK + SPK)
            bulk[:, off : off + SPK] = a0[:, k].reshape(P, SPK)
        for pi, tsel in ((0, range(2, 4)), (1, range(4, 6))):
            ap = stat_arr(tsel)
            base = OFF_P1 + pi * KC * SPK
            for k in range(KC):
                bulk[:, base + k * SPK : base + (k + 1) * SPK] = ap[:, k].reshape(
                    P, SPK
                )
        bulk[:, OFF_B2 : OFF_B2 + STB] = stat_arr(range(6, 12)).reshape(P, STB)
        bulk[:, OFF_B3 :] = stat_arr(range(12, 14)).reshape(P, KC * ST3K)
        # art_eff tiles + host corrections
        tiles = []
        corr = 0.0
        for (bi, bj, jt, W) in jobs:
            I = slice(bi * BLK, (bi + 1) * BLK)
            J = slice(bj * BLK + jt * P, bj * BLK + (jt + 1) * P)
            a_ji = art_dist[J, I].astype(np.float64)        # [128, 512]
            if W == 2:
                a_ij = art_dist[I, J].astype(np.float64).T  # [128, 512]
                abar = 0.5 * (a_ij + a_ji)
                dd = 0.5 * (a_ij - a_ji)
                host["delta"] += 2.0 * float((dd * dd).sum())
            else:
                abar = a_ji
            aef8 = (abar - (s[I] - sbar)[None, :]).astype(f8)
            tiles.append(aef8)
            aef = aef8.astype(np.float64)
            Sa = aef.sum(axis=1)                             # [128]
            Sa2 = (aef * aef).sum(axis=1)
            Sp = qf[J] @ Qsum[bi]                            # sum_i p per j
            cj = s[J] + sbar
            corr += W * float(
                (Sa2 - 2.0 * cj * (2.0 * Sp + Sa) + BLK * cj * cj).sum()
            )
        art = np.concatenate(tiles, axis=1)
        host["cores"].append(corr)
        in_maps.append(
            {"bulk": bulk, "art": np.ascontiguousarray(art)}
        )
    return in_maps, host


def combine(results, host):
    total = host["delta"]
    for c in range(NCORES):
        sp2 = results[c]["sp2"].astype(np.float64)   # [128, 9]
        spa = results[c]["spa"].astype(np.float64)
        dev = 4.0 * (sp2 + spa)
        for pq in range(NPAIR):
            total += PAIR_W[pq] * float(dev[:, pq].sum())
        total += host["cores"][c]
    return np.float32(total / PAIRS)


def _get_nc():
    if "nc" not in _CACHED:
        _CACHED["nc"] = build_graph()
    return _CACHED["nc"]


def _ensure_ntff_hook():
    """The agent image's antenv package lacks axon_hooks, so trace=True in
    run_bass_kernel_spmd crashes on import. Recreate the module + register
    the ctypes NTFF hook the way trn_boot would have."""
    try:
        from antenv.axon_hooks import get_axon_ntff_profile_hook  # noqa: F401

        return
    except ImportError:
        pass
    import types

    import antenv

    mod = types.ModuleType("antenv.axon_hooks")
    holder = {"hook": None}
    mod.set_axon_ntff_profile_hook = lambda h: holder.__setitem__("hook", h)
    mod.get_axon_ntff_profile_hook = lambda: holder["hook"]
    sys.modules["antenv.axon_hooks"] = mod
    antenv.axon_hooks = mod
    try:
        from trn_agent_boot.trn_boot import _ntff_profile_via_ctypes

        for so in ("/opt/axon/libaxon_pjrt.so",):
            if os.path.exists(so):
                holder["hook"] = _ntff_profile_via_ctypes(so)
                break
    except Exception as e:  # degrade: tracing skipped, run still works
        print(f"ntff hook setup failed ({e}); tracing disabled", file=sys.stderr)


def run(embeddings: np.ndarray, art_dist: np.ndarray, **run_kwargs):
    if run_kwargs.get("trace"):
        _ensure_ntff_hook()
    nc = _get_nc()
    in_maps, host = shard_inputs(np.asarray(embeddings), np.asarray(art_dist))
    res = run_bass_kernel_spmd(nc, in_maps, core_ids=list(range(NCORES)), **run_kwargs)
    loss = combine(res.results, host)
    return np.asarray(loss, dtype=np.float32), res


def kernel(embeddings: np.ndarray, art_dist: np.ndarray) -> np.ndarray:
    loss, _ = run(embeddings, art_dist)
    return loss
